# revision 13
# baseline (speedup 1.0000x reference)
"""Trainium2 Bass kernel for nn_ChemResBlock (gnn_message_passing).

Reference computation (A=2048 atoms, D=64 depth, F=12 filter slots):
    chemconv(x)[a,o] = sum_{n,f,d} conn[a,n,f] * x[n,d] * filters[o,f,d]
                       + sum_{f,c} bond[a,f,c] * filters[o,f,D+c]
    for filt in (f0, f1):
        out = relu(chemconv(out)); out = chemconv(out); out = relu(out + x)

Kernel strategy (8 NeuronCores), v2:
  * Contract-reorder: out[a,o] = sum_{n,f} conn[a,n,f] * y[n,f,o] with
    y[n,f,o] = sum_d x[n,d]*filters[o,f,d]  (tiny per-shard precompute), so
    the big conn tensor is consumed by plain [128,64]x[128,512] matmuls.
  * Shard the contraction (neighbor) dim n across 8 cores (256 each).
  * conn is fed to the device in fp16 (host cast): the 25MB/core f32 shard
    becomes 12.6MB, which is BOTH half the HBM traffic (memory-roofline
    regime) AND small enough to keep the whole shard resident in SBUF, so
    conn is read from HBM exactly once.  All matmul operands are fp16
    (~tf32-precision multiplies, f32 psum accumulation).
  * Per conv the partial z^T [64, 2048] accumulates in 4 psum banks.  conn
    columns are host-permuted to [half, rank, local128] so banks {0,1} hold
    every rank's local columns 0-127 ("half A") and banks {2,3} hold local
    columns 128-255 ("half B").  Each half is reduced by its own fp16
    8-rank ReduceScatter as soon as it is drained, so the collective for
    half A flies while the PE computes half B, and the next conv's matmuls
    for ns0-chunks overlap the half-B collective.
  * conv0 runs a single chunk-outer pass so it consumes conn chunks in DMA
    arrival order (conv0 is HBM-bound; later convs are PE-bound).
  * A dummy warm-up ReduceScatter is fired during setup to absorb the
    first-collective initialization cost off the critical path.
  * Final output is per-core [64, 256] f32 shards, concatenated + transposed
    on the host (pure layout).
"""

import os

import numpy as np

import concourse.bacc as bacc
import concourse.bass as bass
import concourse.mybir as mybir
import concourse.tile as tile
from concourse.bass_utils import run_bass_kernel_spmd

A, D, F, NCORES = 2048, 64, 12, 8
NS = A // NCORES          # neighbors per core = 256
KL = NS * F               # local contraction size = 3072
NCH = KL // 128           # k-chunks of 128 = 24
FO = F * D                # 768 = y columns per layer

FP = mybir.dt.float32
HP = mybir.dt.float16

# Tunables (env-overridable for experiments)
WARMCC = os.environ.get("CHEM_WARMCC", "1") == "1"
SPLIT = os.environ.get("CHEM_SPLIT", "1") == "1"   # 2 RS per conv vs 1

# Intermediate values grow ~40x per conv (up to ~5.7e8 by conv3), far past
# fp16 range.  relu is positively homogeneous, so we carry an exact
# power-of-2 scale on the value path: the conv-c drain multiplies by
# SCALE[c] (cumulative CUM[c]), bias/residual operands are pre-scaled to
# match, and the final f32 relu multiplies by 1/CUM[3].  Headroom vs the
# measured maxima is >3x everywhere.
SCALE = [1.0, 1.0, 2.0 ** -7, 2.0 ** -8]
CUM = [1.0, 1.0, 2.0 ** -7, 2.0 ** -15]

_CACHE = {}

EVENS = list(range(0, NCH, 2))   # chunks with ns=0 (local cols 0-127)
ODDS = list(range(1, NCH, 2))    # chunks with ns=1


def _build():
    nc = bacc.Bacc("TRN2", target_bir_lowering=False, debug=False, num_devices=NCORES)

    conn_t_d = nc.dram_tensor("conn_t", [KL, A], HP, kind="ExternalInput").ap()
    xoT_d = nc.dram_tensor("xoT_sh", [D, NS], HP, kind="ExternalInput").ap()
    fw_d = nc.dram_tensor("fw", [D, 2 * FO], HP, kind="ExternalInput").ap()
    fb_d = nc.dram_tensor("fb", [2 * F, 2 * D], HP, kind="ExternalInput").ap()
    bondT_d = nc.dram_tensor("bondT_sh", [2 * F, NS], HP, kind="ExternalInput").ap()
    out_d = nc.dram_tensor("out_sh", [D, NS], FP, kind="ExternalOutput").ap()

    with tile.TileContext(nc) as tc:
        with (
            tc.tile_pool(name="res", bufs=1) as res_pool,
            tc.tile_pool(name="sb", bufs=1) as sb,
            tc.tile_pool(name="ypool", bufs=1) as ypool,
            tc.tile_pool(name="ztpool", bufs=1) as ztpool,
            tc.tile_pool(name="work", bufs=2) as work,
            tc.tile_pool(name="curp", bufs=1) as curp,
            tc.tile_pool(name="psy", bufs=2, space="PSUM") as psy,
            tc.tile_pool(name="psz", bufs=1, space="PSUM") as psz,
            tc.tile_pool(name="dram", bufs=1, space="DRAM") as dram,
        ):
            # ---- setup: small tensors first (ACT HWDGE ring) ----
            xoT_sb = sb.tile([D, NS], HP, name="xoT_sb", tag="xoT_sb")
            nc.scalar.dma_start(xoT_sb[:], xoT_d)
            fw_sb = sb.tile([D, 2 * FO], HP, name="fw_sb", tag="fw_sb")
            nc.scalar.dma_start(fw_sb[:], fw_d)
            fb_sb = sb.tile([2 * F, 2 * D], HP, name="fb_sb", tag="fb_sb")
            nc.scalar.dma_start(fb_sb[:], fb_d)
            bondT_sb = sb.tile([2 * F, NS], HP, name="bondT_sb", tag="bondT_sb")
            nc.scalar.dma_start(bondT_sb[:], bondT_d)

            # warm-up collective: absorbs first-CC init cost during the
            # (DMA-bound) conn load.  Data is junk; result unused.  Must be
            # issued BEFORE the conn bulk DMAs so its input doesn't queue
            # behind 6MB of conn on the ACT ring.
            if WARMCC:
                wu_in = dram.tile([NCORES, 16], HP, name="wu_in", tag="wu_in")
                wu_out = dram.tile([16], HP, name="wu_out", tag="wu_out")
                nc.scalar.dma_start(wu_in[:], xoT_sb[0:NCORES, 0:16])
                nc.gpsimd.collective_compute(
                    "ReduceScatter",
                    mybir.AluOpType.add,
                    replica_groups=[list(range(NCORES))],
                    ins=[wu_in.opt()],
                    outs=[wu_out.opt()],
                )

            # conn chunks split across SP + ACT rings (HBM-bound phase)
            conn_res = []
            for r in range(NCH):
                t = res_pool.tile([128, A], HP, name=f"connsb{r}", tag=f"connsb{r}")
                eng = nc.sync if r % 2 == 0 else nc.scalar
                eng.dma_start(t[:], conn_t_d[r * 128:(r + 1) * 128, :])
                conn_res.append(t)

            # bias shards, pre-scaled per conv: slot c holds bias[layer(c)]*CUM[c]
            bias16 = sb.tile([D, 4, NS], HP, name="bias16", tag="bias16")
            for layer in range(2):
                pb = psy.tile([D, NS], FP, name="pb", tag="py")
                nc.tensor.matmul(
                    pb[:], fb_sb[:, layer * D:(layer + 1) * D], bondT_sb[:],
                    start=True, stop=True,
                )
                for conv in (2 * layer, 2 * layer + 1):
                    if CUM[conv] == 1.0:
                        nc.vector.tensor_copy(bias16[:, conv, :], pb[:])
                    else:
                        nc.vector.tensor_scalar_mul(
                            bias16[:, conv, :], pb[:], CUM[conv])

            # residual x for conv3, pre-scaled to CUM[3], kept f32
            x3_32 = sb.tile([D, NS], FP, name="x3_32", tag="x3_32")
            nc.vector.tensor_scalar_mul(x3_32[:], xoT_sb[:], CUM[3])

            # collective buffers (per conv, per half)
            cc_in = [[dram.tile([NCORES, D, 128], HP, name=f"cc_in{c}{h}",
                                tag=f"cc_in{c}{h}")
                      for h in range(2)] for c in range(4)]
            cc_out = [[dram.tile([D, 128], HP, name=f"cc_out{c}{h}",
                                 tag=f"cc_out{c}{h}")
                       for h in range(2)] for c in range(4)]

            # persistent z psum banks: [half][bank-within-half]
            pz = [[psz.tile([D, 512], FP, name=f"pz{h}{b}", tag=f"pz{h}{b}")
                   for b in range(2)] for h in range(2)]

            # y tiles: y[ns][n_local128, (f, o)]
            y_sb = [ypool.tile([128, FO], HP, name=f"y{ns}_sb", tag=f"y{ns}_sb")
                    for ns in range(2)]

            def produce_y(src16, layer, ns):
                """y[n,f,o] for one 128-col half. src16: [64(d), 128(n)] fp16."""
                for h in range(2):
                    py = psy.tile([128, FO // 2], FP, name="py", tag="py")
                    nc.tensor.matmul(
                        py[:], src16,
                        fw_sb[:, layer * FO + h * (FO // 2):
                              layer * FO + (h + 1) * (FO // 2)],
                        start=True, stop=True,
                    )
                    nc.vector.tensor_copy(
                        y_sb[ns][:, h * (FO // 2):(h + 1) * (FO // 2)], py[:]
                    )

            def lhs_for(r):
                return y_sb[r % 2][:, (r // 2) * D:(r // 2 + 1) * D]

            def mm_half(h, order, first, last):
                """Accumulate both banks of half h over the given chunk order.
                first/last: whether this call begins/ends the accumulation."""
                for idx, r in enumerate(order):
                    lhsT = lhs_for(r)
                    for b in range(2):
                        c0 = h * 1024 + b * 512
                        nc.tensor.matmul(
                            pz[h][b][:], lhsT, conn_res[r][:, c0:c0 + 512],
                            start=(first and idx == 0),
                            stop=(last and idx == len(order) - 1),
                        )

            def drain_fire(conv, h):
                """Cast half-h banks to fp16, ship to DRAM, fire its RS."""
                # cast+scale on the ACT engine (tensor op, not its DMA ring)
                # and ship on the SP ring: keeps the DVE queue (consume/y
                # path) and the ACT ring (consume readbacks) clear.
                zt = ztpool.tile([D, 1024], HP, name=f"zt{h}", tag=f"zt{h}")
                for b in range(2):
                    nc.scalar.activation(
                        zt[:, b * 512:(b + 1) * 512], pz[h][b][:],
                        mybir.ActivationFunctionType.Copy, scale=SCALE[conv])
                for blk in range(NCORES):
                    nc.sync.dma_start(
                        cc_in[conv][h][blk, :, :], zt[:, blk * 128:(blk + 1) * 128]
                    )
                nc.gpsimd.collective_compute(
                    "ReduceScatter",
                    mybir.AluOpType.add,
                    replica_groups=[list(range(NCORES))],
                    ins=[cc_in[conv][h].opt()],
                    outs=[cc_out[conv][h].opt()],
                )

            def consume(conv, h):
                """Read reduced half, apply bias(+residual)+relu -> cur fp16.

                Everything on the value path carries the exact CUM[conv]
                scale; conv3 runs in f32 and multiplies by 1/CUM[3]."""
                hs = slice(h * 128, (h + 1) * 128)
                sl = work.tile([D, 128], HP, name=f"sl{h}", tag=f"sl{h}")
                nc.scalar.dma_start(sl[:], cc_out[conv][h])
                if conv == 3:
                    t1 = work.tile([D, 128], FP, name=f"t1f{h}", tag=f"t1f{h}")
                    nc.vector.tensor_add(t1[:], sl[:], bias16[:, conv, hs])
                    t2 = work.tile([D, 128], FP, name=f"t2f{h}", tag=f"t2f{h}")
                    nc.vector.tensor_add(t2[:], t1[:], x3_32[:, hs])
                    o32 = work.tile([D, 128], FP, name=f"o32{h}", tag=f"o32{h}")
                    nc.vector.tensor_scalar(
                        o32[:], t2[:], 0.0, 1.0 / CUM[3],
                        mybir.AluOpType.max, mybir.AluOpType.mult)
                    nc.scalar.dma_start(out_d[:, hs], o32[:])
                    return None
                t1 = work.tile([D, 128], HP, name=f"t1{h}", tag=f"t1{h}")
                nc.vector.tensor_add(t1[:], sl[:], bias16[:, conv, hs])
                if conv % 2 == 1:
                    t2 = work.tile([D, 128], HP, name=f"t2{h}", tag=f"t2{h}")
                    nc.vector.tensor_add(t2[:], t1[:], xoT_sb[:, hs])
                    t1 = t2
                cur = curp.tile([D, 128], HP, name=f"cur{conv}{h}",
                                tag=f"cur{conv % 2}{h}")
                nc.vector.tensor_scalar_max(cur[:], t1[:], 0.0)
                return cur

            scope = nc.named_scope

            # ---- conv0: chunk-outer single pass (DMA arrival order) ----
            sc = scope("conv0"); sc.__enter__()
            produce_y(xoT_sb[:, 0:128], 0, 0)
            produce_y(xoT_sb[:, 128:256], 0, 1)
            for r in range(NCH):
                lhsT = lhs_for(r)
                for h in range(2):
                    for b in range(2):
                        c0 = h * 1024 + b * 512
                        nc.tensor.matmul(
                            pz[h][b][:], lhsT, conn_res[r][:, c0:c0 + 512],
                            start=(r == 0), stop=(r == NCH - 1),
                        )
            drain_fire(0, 0)
            drain_fire(0, 1)
            sc.__exit__(None, None, None)

            # ---- convs 1-3: half-outer with pipelined per-half RS ----
            for conv in range(1, 4):
                sc = scope(f"conv{conv}"); sc.__enter__()
                layer = conv // 2
                curA = consume(conv - 1, 0)
                produce_y(curA[:], layer, 0)
                mm_half(0, EVENS, True, False)   # y[0] only; RS_B in flight
                curB = consume(conv - 1, 1)
                produce_y(curB[:], layer, 1)
                mm_half(0, ODDS, False, True)
                drain_fire(conv, 0)
                mm_half(1, EVENS + ODDS, True, True)
                drain_fire(conv, 1)
                sc.__exit__(None, None, None)

            # ---- final halves: bias + residual + relu -> out ----
            sc = scope("fin"); sc.__enter__()
            consume(3, 0)
            consume(3, 1)
            sc.__exit__(None, None, None)

    nc.compile()
    return nc


def _get_nc():
    if "nc" not in _CACHE:
        _CACHE["nc"] = _build()
    return _CACHE["nc"]


def _prep_core(conn, xT, fw16, fb16, bondT16, c):
    sl = slice(c * NS, (c + 1) * NS)
    # rows (f, n_local); cols host-permuted [half, rank, local128]
    ct = conn[:, sl, :].astype(np.float16)           # [A, NS, F]
    ct = ct.reshape(NCORES, 2, 128, NS, F)
    ct = np.ascontiguousarray(ct.transpose(4, 3, 1, 0, 2).reshape(KL, A))
    return {
        "conn_t": ct,
        "xoT_sh": np.ascontiguousarray(xT[:, sl]).astype(np.float16),
        "fw": fw16,
        "fb": fb16,
        "bondT_sh": np.ascontiguousarray(bondT16[:, sl]),
    }


def make_in_maps(**inputs):
    """Host-side prep (pure layout transforms + fp16 cast)."""
    x = np.ascontiguousarray(inputs["node_property_tensor"], dtype=np.float32)
    conn = np.ascontiguousarray(inputs["connectivity_tensor"], dtype=np.float32)
    bond = np.ascontiguousarray(inputs["bond_property_tensor"], dtype=np.float32)
    f0 = np.ascontiguousarray(inputs["filters0"], dtype=np.float32)
    f1 = np.ascontiguousarray(inputs["filters1"], dtype=np.float32)
    xT = np.ascontiguousarray(x.T)
    fw16 = np.ascontiguousarray(np.concatenate(
        [f[:, :, :D].transpose(2, 1, 0).reshape(D, FO) for f in (f0, f1)],
        axis=1)).astype(np.float16)
    fb16 = np.ascontiguousarray(np.concatenate(
        [f[:, :, D:].reshape(D, 2 * F).T for f in (f0, f1)],
        axis=1)).astype(np.float16)
    bondT16 = np.ascontiguousarray(
        bond.transpose(1, 2, 0).reshape(2 * F, A)).astype(np.float16)
    return [_prep_core(conn, xT, fw16, fb16, bondT16, c) for c in range(NCORES)]


def kernel(node_property_tensor, connectivity_tensor, bond_property_tensor,
           filters0, filters1):
    in_maps = make_in_maps(
        node_property_tensor=node_property_tensor,
        connectivity_tensor=connectivity_tensor,
        bond_property_tensor=bond_property_tensor,
        filters0=filters0,
        filters1=filters1,
    )
    nc = _get_nc()
    res = run_bass_kernel_spmd(nc, in_maps, core_ids=list(range(NCORES)))
    outT = np.concatenate([res.results[c]["out_sh"] for c in range(NCORES)], axis=1)
    return np.ascontiguousarray(outT.T)


def run_traced(in_maps, stitch=False):
    """For test.py: run with NTFF tracing, return BassKernelResults."""
    kw = {}
    if stitch:
        kw = dict(trace_cores=list(range(NCORES)), stitch_traces=True)
    return run_bass_kernel_spmd(
        _get_nc(), in_maps, core_ids=list(range(NCORES)), trace=True, **kw
    )


# revision 16
# speedup vs baseline: 1.0846x; 1.0846x over previous
"""Trainium2 Bass kernel for nn_ChemResBlock (gnn_message_passing).

Reference computation (A=2048 atoms, D=64 depth, F=12 filter slots):
    chemconv(x)[a,o] = sum_{n,f,d} conn[a,n,f] * x[n,d] * filters[o,f,d]
                       + sum_{f,c} bond[a,f,c] * filters[o,f,D+c]
    for filt in (f0, f1):
        out = relu(chemconv(out)); out = chemconv(out); out = relu(out + x)

Kernel strategy (8 NeuronCores), v2:
  * Contract-reorder: out[a,o] = sum_{n,f} conn[a,n,f] * y[n,f,o] with
    y[n,f,o] = sum_d x[n,d]*filters[o,f,d]  (tiny per-shard precompute), so
    the big conn tensor is consumed by plain [128,64]x[128,512] matmuls.
  * Shard the contraction (neighbor) dim n across 8 cores (256 each).
  * conn is fed to the device in fp16 (host cast): the 25MB/core f32 shard
    becomes 12.6MB, which is BOTH half the HBM traffic (memory-roofline
    regime) AND small enough to keep the whole shard resident in SBUF, so
    conn is read from HBM exactly once.  All matmul operands are fp16
    (~tf32-precision multiplies, f32 psum accumulation).
  * Per conv the partial z^T [64, 2048] accumulates in 4 psum banks.  conn
    columns are host-permuted to [half, rank, local128] so banks {0,1} hold
    every rank's local columns 0-127 ("half A") and banks {2,3} hold local
    columns 128-255 ("half B").  Each half is reduced by its own fp16
    8-rank ReduceScatter as soon as it is drained, so the collective for
    half A flies while the PE computes half B, and the next conv's matmuls
    for ns0-chunks overlap the half-B collective.
  * conv0 runs a single chunk-outer pass so it consumes conn chunks in DMA
    arrival order (conv0 is HBM-bound; later convs are PE-bound).
  * A dummy warm-up ReduceScatter is fired during setup to absorb the
    first-collective initialization cost off the critical path.
  * Final output is per-core [64, 256] f32 shards, concatenated + transposed
    on the host (pure layout).
"""

import os

import numpy as np

import concourse.bacc as bacc
import concourse.bass as bass
import concourse.mybir as mybir
import concourse.tile as tile
from concourse.bass_utils import run_bass_kernel_spmd

A, D, F, NCORES = 2048, 64, 12, 8
NS = A // NCORES          # neighbors per core = 256
KL = NS * F               # local contraction size = 3072
NCH = KL // 128           # k-chunks of 128 = 24
FO = F * D                # 768 = y columns per layer

FP = mybir.dt.float32
HP = mybir.dt.float16

# Tunables (env-overridable for experiments)
WARMCC = os.environ.get("CHEM_WARMCC", "1") == "1"
SPLIT = os.environ.get("CHEM_SPLIT", "1") == "1"   # 2 RS per conv vs 1

# Intermediate values grow ~40x per conv (up to ~5.7e8 by conv3), far past
# fp16 range.  relu is positively homogeneous, so we carry an exact
# power-of-2 scale on the value path: the conv-c drain multiplies by
# SCALE[c] (cumulative CUM[c]), bias/residual operands are pre-scaled to
# match, and the final f32 relu multiplies by 1/CUM[3].  Headroom vs the
# measured maxima is >3x everywhere.
SCALE = [1.0, 1.0, 2.0 ** -7, 2.0 ** -8]
CUM = [1.0, 1.0, 2.0 ** -7, 2.0 ** -15]

_CACHE = {}

EVENS = list(range(0, NCH, 2))   # chunks with ns=0 (local cols 0-127)
ODDS = list(range(1, NCH, 2))    # chunks with ns=1


def _build():
    nc = bacc.Bacc("TRN2", target_bir_lowering=False, debug=False, num_devices=NCORES)

    conn_t_d = nc.dram_tensor("conn_t", [KL, A], HP, kind="ExternalInput").ap()
    xoT_d = nc.dram_tensor("xoT_sh", [D, NS], HP, kind="ExternalInput").ap()
    fw_d = nc.dram_tensor("fw", [D, 2 * FO], HP, kind="ExternalInput").ap()
    fb_d = nc.dram_tensor("fb", [2 * F, 2 * D], HP, kind="ExternalInput").ap()
    bondT_d = nc.dram_tensor("bondT_sh", [2 * F, NS], HP, kind="ExternalInput").ap()
    out_d = nc.dram_tensor("out_sh", [D, NS], FP, kind="ExternalOutput").ap()

    with tile.TileContext(nc) as tc:
        with (
            tc.tile_pool(name="res", bufs=1) as res_pool,
            tc.tile_pool(name="sb", bufs=1) as sb,
            tc.tile_pool(name="ypool", bufs=1) as ypool,
            tc.tile_pool(name="ztpool", bufs=1) as ztpool,
            tc.tile_pool(name="work", bufs=2) as work,
            tc.tile_pool(name="curp", bufs=1) as curp,
            tc.tile_pool(name="psy", bufs=2, space="PSUM") as psy,
            tc.tile_pool(name="psz", bufs=1, space="PSUM") as psz,
            tc.tile_pool(name="dram", bufs=1, space="DRAM") as dram,
        ):
            # ---- setup: small tensors first, split across both rings ----
            xoT_sb = sb.tile([D, NS], HP, name="xoT_sb", tag="xoT_sb")
            nc.sync.dma_start(xoT_sb[:], xoT_d)
            fw_sb = sb.tile([D, 2 * FO], HP, name="fw_sb", tag="fw_sb")
            nc.scalar.dma_start(fw_sb[:], fw_d)
            fb_sb = sb.tile([2 * F, 2 * D], HP, name="fb_sb", tag="fb_sb")
            nc.sync.dma_start(fb_sb[:], fb_d)
            bondT_sb = sb.tile([2 * F, NS], HP, name="bondT_sb", tag="bondT_sb")
            nc.scalar.dma_start(bondT_sb[:], bondT_d)

            # warm-up collective: absorbs first-CC init cost during the
            # (DMA-bound) conn load.  Data is junk; result unused.  Must be
            # issued BEFORE the conn bulk DMAs so its input doesn't queue
            # behind 6MB of conn on the ACT ring.
            if WARMCC:
                wu_in = dram.tile([NCORES, 16], HP, name="wu_in", tag="wu_in")
                wu_out = dram.tile([16], HP, name="wu_out", tag="wu_out")
                nc.scalar.dma_start(wu_in[:], xoT_sb[0:NCORES, 0:16])
                nc.gpsimd.collective_compute(
                    "ReduceScatter",
                    mybir.AluOpType.add,
                    replica_groups=[list(range(NCORES))],
                    ins=[wu_in.opt()],
                    outs=[wu_out.opt()],
                )

            # conn chunks split across SP + ACT rings (HBM-bound phase)
            conn_res = []
            for r in range(NCH):
                t = res_pool.tile([128, A], HP, name=f"connsb{r}", tag=f"connsb{r}")
                eng = nc.sync if r % 2 == 0 else nc.scalar
                eng.dma_start(t[:], conn_t_d[r * 128:(r + 1) * 128, :])
                conn_res.append(t)

            # bias shards, pre-scaled per conv: slot c holds bias[layer(c)]*CUM[c]
            bias16 = sb.tile([D, 4, NS], HP, name="bias16", tag="bias16")
            for layer in range(2):
                pb = psy.tile([D, NS], FP, name="pb", tag="py")
                nc.tensor.matmul(
                    pb[:], fb_sb[:, layer * D:(layer + 1) * D], bondT_sb[:],
                    start=True, stop=True,
                )
                for conv in (2 * layer, 2 * layer + 1):
                    if CUM[conv] == 1.0:
                        nc.vector.tensor_copy(bias16[:, conv, :], pb[:])
                    else:
                        nc.vector.tensor_scalar_mul(
                            bias16[:, conv, :], pb[:], CUM[conv])

            # residual x for conv3, pre-scaled to CUM[3], kept f32
            x3_32 = sb.tile([D, NS], FP, name="x3_32", tag="x3_32")
            nc.vector.tensor_scalar_mul(x3_32[:], xoT_sb[:], CUM[3])

            # collective buffers (per conv, per half)
            cc_in = [[dram.tile([NCORES, D, 128], HP, name=f"cc_in{c}{h}",
                                tag=f"cc_in{c}{h}")
                      for h in range(2)] for c in range(4)]
            cc_out = [[dram.tile([D, 128], HP, name=f"cc_out{c}{h}",
                                 tag=f"cc_out{c}{h}")
                       for h in range(2)] for c in range(4)]

            # persistent z psum banks: [half][bank-within-half]
            pz = [[psz.tile([D, 512], FP, name=f"pz{h}{b}", tag=f"pz{h}{b}")
                   for b in range(2)] for h in range(2)]

            # y tiles: y[ns][n_local128, (f, o)]
            y_sb = [ypool.tile([128, FO], HP, name=f"y{ns}_sb", tag=f"y{ns}_sb")
                    for ns in range(2)]

            def produce_y(src16, layer, ns):
                """y[n,f,o] for one 128-col half. src16: [64(d), 128(n)] fp16."""
                for h in range(2):
                    py = psy.tile([128, FO // 2], FP, name="py", tag="py")
                    nc.tensor.matmul(
                        py[:], src16,
                        fw_sb[:, layer * FO + h * (FO // 2):
                              layer * FO + (h + 1) * (FO // 2)],
                        start=True, stop=True,
                    )
                    nc.vector.tensor_copy(
                        y_sb[ns][:, h * (FO // 2):(h + 1) * (FO // 2)], py[:]
                    )

            def lhs_for(r):
                return y_sb[r % 2][:, (r // 2) * D:(r // 2 + 1) * D]

            def mm_half(h, order, first, last):
                """Accumulate both banks of half h over the given chunk order.
                first/last: whether this call begins/ends the accumulation."""
                for idx, r in enumerate(order):
                    lhsT = lhs_for(r)
                    for b in range(2):
                        c0 = h * 1024 + b * 512
                        nc.tensor.matmul(
                            pz[h][b][:], lhsT, conn_res[r][:, c0:c0 + 512],
                            start=(first and idx == 0),
                            stop=(last and idx == len(order) - 1),
                        )

            def drain_fire(conv, h):
                """Cast half-h banks to fp16, ship to DRAM, fire its RS."""
                # cast+scale on DVE; ship with ONE rank-major DMA on the SP
                # ring (rearranged DRAM dst), so the RS fires ~1us after the
                # banks complete instead of behind 8 serial DMA issues.
                zt = ztpool.tile([D, 1024], HP, name=f"zt{h}", tag=f"zt{h}")
                for b in range(2):
                    if SCALE[conv] == 1.0:
                        nc.vector.tensor_copy(
                            zt[:, b * 512:(b + 1) * 512], pz[h][b][:])
                    else:
                        nc.vector.tensor_scalar_mul(
                            zt[:, b * 512:(b + 1) * 512], pz[h][b][:], SCALE[conv])
                nc.sync.dma_start(
                    cc_in[conv][h].rearrange("r o l -> o r l"), zt[:])
                nc.gpsimd.collective_compute(
                    "ReduceScatter",
                    mybir.AluOpType.add,
                    replica_groups=[list(range(NCORES))],
                    ins=[cc_in[conv][h].opt()],
                    outs=[cc_out[conv][h].opt()],
                )

            def consume(conv, h):
                """Read reduced half, apply bias(+residual)+relu -> cur fp16.

                Everything on the value path carries the exact CUM[conv]
                scale; conv3 runs in f32 and multiplies by 1/CUM[3]."""
                hs = slice(h * 128, (h + 1) * 128)
                sl = work.tile([D, 128], HP, name=f"sl{h}", tag=f"sl{h}")
                nc.scalar.dma_start(sl[:], cc_out[conv][h])
                if conv == 3:
                    t1 = work.tile([D, 128], FP, name=f"t1f{h}", tag=f"t1f{h}")
                    nc.vector.tensor_add(t1[:], sl[:], bias16[:, conv, hs])
                    t2 = work.tile([D, 128], FP, name=f"t2f{h}", tag=f"t2f{h}")
                    nc.vector.tensor_add(t2[:], t1[:], x3_32[:, hs])
                    o32 = work.tile([D, 128], FP, name=f"o32{h}", tag=f"o32{h}")
                    nc.vector.tensor_scalar(
                        o32[:], t2[:], 0.0, 1.0 / CUM[3],
                        mybir.AluOpType.max, mybir.AluOpType.mult)
                    nc.scalar.dma_start(out_d[:, hs], o32[:])
                    return None
                t1 = work.tile([D, 128], HP, name=f"t1{h}", tag=f"t1{h}")
                nc.vector.tensor_add(t1[:], sl[:], bias16[:, conv, hs])
                if conv % 2 == 1:
                    t2 = work.tile([D, 128], HP, name=f"t2{h}", tag=f"t2{h}")
                    nc.vector.tensor_add(t2[:], t1[:], xoT_sb[:, hs])
                    t1 = t2
                cur = curp.tile([D, 128], HP, name=f"cur{conv}{h}",
                                tag=f"cur{conv % 2}{h}")
                nc.vector.tensor_scalar_max(cur[:], t1[:], 0.0)
                return cur

            scope = nc.named_scope

            # ---- conv0: two passes over the (streaming) chunks.  Pass A is
            # DMA-paced; its RS fires ~15us before the resident-data pass B
            # finishes, overlapping the first collective with compute. ----
            sc = scope("conv0"); sc.__enter__()
            produce_y(xoT_sb[:, 0:128], 0, 0)
            produce_y(xoT_sb[:, 128:256], 0, 1)
            mm_half(0, list(range(NCH)), True, True)
            drain_fire(0, 0)
            mm_half(1, list(range(NCH)), True, True)
            drain_fire(0, 1)
            sc.__exit__(None, None, None)

            # ---- convs 1-3: half-outer with pipelined per-half RS ----
            for conv in range(1, 4):
                sc = scope(f"conv{conv}"); sc.__enter__()
                layer = conv // 2
                curA = consume(conv - 1, 0)
                produce_y(curA[:], layer, 0)
                mm_half(0, EVENS, True, False)   # y[0] only; RS_B in flight
                curB = consume(conv - 1, 1)
                produce_y(curB[:], layer, 1)
                mm_half(0, ODDS, False, True)
                drain_fire(conv, 0)
                mm_half(1, EVENS + ODDS, True, True)
                drain_fire(conv, 1)
                sc.__exit__(None, None, None)

            # ---- final halves: bias + residual + relu -> out ----
            sc = scope("fin"); sc.__enter__()
            consume(3, 0)
            consume(3, 1)
            sc.__exit__(None, None, None)

    nc.compile()
    return nc


def _get_nc():
    if "nc" not in _CACHE:
        _CACHE["nc"] = _build()
    return _CACHE["nc"]


def _prep_core(conn, xT, fw16, fb16, bondT16, c):
    sl = slice(c * NS, (c + 1) * NS)
    # rows (f, n_local); cols host-permuted [half, rank, local128]
    ct = conn[:, sl, :].astype(np.float16)           # [A, NS, F]
    ct = ct.reshape(NCORES, 2, 128, NS, F)
    ct = np.ascontiguousarray(ct.transpose(4, 3, 1, 0, 2).reshape(KL, A))
    return {
        "conn_t": ct,
        "xoT_sh": np.ascontiguousarray(xT[:, sl]).astype(np.float16),
        "fw": fw16,
        "fb": fb16,
        "bondT_sh": np.ascontiguousarray(bondT16[:, sl]),
    }


def make_in_maps(**inputs):
    """Host-side prep (pure layout transforms + fp16 cast)."""
    x = np.ascontiguousarray(inputs["node_property_tensor"], dtype=np.float32)
    conn = np.ascontiguousarray(inputs["connectivity_tensor"], dtype=np.float32)
    bond = np.ascontiguousarray(inputs["bond_property_tensor"], dtype=np.float32)
    f0 = np.ascontiguousarray(inputs["filters0"], dtype=np.float32)
    f1 = np.ascontiguousarray(inputs["filters1"], dtype=np.float32)
    xT = np.ascontiguousarray(x.T)
    fw16 = np.ascontiguousarray(np.concatenate(
        [f[:, :, :D].transpose(2, 1, 0).reshape(D, FO) for f in (f0, f1)],
        axis=1)).astype(np.float16)
    fb16 = np.ascontiguousarray(np.concatenate(
        [f[:, :, D:].reshape(D, 2 * F).T for f in (f0, f1)],
        axis=1)).astype(np.float16)
    bondT16 = np.ascontiguousarray(
        bond.transpose(1, 2, 0).reshape(2 * F, A)).astype(np.float16)
    return [_prep_core(conn, xT, fw16, fb16, bondT16, c) for c in range(NCORES)]


def kernel(node_property_tensor, connectivity_tensor, bond_property_tensor,
           filters0, filters1):
    in_maps = make_in_maps(
        node_property_tensor=node_property_tensor,
        connectivity_tensor=connectivity_tensor,
        bond_property_tensor=bond_property_tensor,
        filters0=filters0,
        filters1=filters1,
    )
    nc = _get_nc()
    res = run_bass_kernel_spmd(nc, in_maps, core_ids=list(range(NCORES)))
    outT = np.concatenate([res.results[c]["out_sh"] for c in range(NCORES)], axis=1)
    return np.ascontiguousarray(outT.T)


def run_traced(in_maps, stitch=False):
    """For test.py: run with NTFF tracing, return BassKernelResults."""
    kw = {}
    if stitch:
        kw = dict(trace_cores=list(range(NCORES)), stitch_traces=True)
    return run_bass_kernel_spmd(
        _get_nc(), in_maps, core_ids=list(range(NCORES)), trace=True, **kw
    )
